# revision 8
# baseline (speedup 1.0000x reference)
"""Trainium2 Bass kernel for nn_Mix8Net (dense directional-conv CNN).

Data-parallel over 8 NeuronCores: batch 1024 -> 128 samples/core.

Per-core dataflow (channels on SBUF partitions, pixels*samples on free dim):
  - Activations live in a "tall" layout: per-sample block = 16 rows x 16
    cols = 256 fp32 (row 0 = zero separator, col 0 = zero border, 15x15
    image at rows 1-15 / cols 1-15).  A 3-tap directional conv tap with
    offset (dy,dx) is then a matmul whose moving operand is the
    per-sample flat range [block+16, block+256) shifted by 16*dy+dx —
    out-of-image reads land on the zero separators/borders.
  - All matmuls run in float32r (TF32-like, 1 cycle/row on the PE,
    ~1.5e-4 relative error) accumulating fp32 in PSUM.  fp32r requires
    even innermost AP counts, hence the 240-wide (15 rows x 16 cols)
    moving ranges; the extra border column produces garbage output
    columns that stay column-isolated through the whole network and are
    stripped on the final DMA.
  - Silu runs on ScalarE straight out of PSUM (4 banks / 1920 cols per
    instruction); residual adds run on VectorE over interior-only 4D
    views so the zero borders of the tall buffers are never corrupted.
  - The initial conv (CIN=2, 3 taps) is packed into one K=6 matmul per
    sample pair using host-prepared pre-shifted copies of x, placed at
    partitions 32*d (one group per direction).
"""
import numpy as np
import concourse.bacc as bacc
import concourse.mybir as mybir
import concourse.tile as tile
from concourse import bass_utils

F32 = mybir.dt.float32
F32R = mybir.dt.float32r
AF = mybir.ActivationFunctionType

DIR_OFFSETS = (
    ((0, -1), (0, 0), (0, 1)),
    ((-1, 0), (0, 0), (1, 0)),
    ((-1, -1), (0, 0), (1, 1)),
    ((1, -1), (0, 0), (-1, 1)),
)

B, CIN, H, W = 1024, 2, 15, 15
M, COUT = 128, 64
NB = 4                      # DirectionalConvResBlocks
NCORES = 8
BLOC = B // NCORES          # 128 samples per core
NS = 16                     # samples per chunk
NCH = BLOC // NS            # 8 chunks
BLK = 256                   # 16x16 per-sample block
TALLB = (NS + 2) * BLK      # chunk window incl lead/trail blocks
TOT = (BLOC + 2) * BLK      # full-core tall array length

_CACHE = {}


def _flat240(buf, s0, nsamp, delta=0, psl=None):
    """[p, nsamp(256), 240] moving/store view starting at sample s0."""
    a = 256 * (s0 + 1) + 16 + delta
    v = buf[:, a:a + nsamp * 256] if psl is None else buf[psl, a:a + nsamp * 256]
    return v.rearrange("p (s q) -> p s q", q=BLK)[:, :, 0:240]


def _interior(buf):
    """[p, NS(256), 15(16), 15(1)] interior-pixel view (all NS samples)."""
    a = BLK + 17
    v = buf[:, a:a + NS * BLK]
    v = v.rearrange("p (s q) -> p s q", q=BLK)[:, :, 0:240]
    return v.rearrange("p s (r c) -> p s r c", c=16)[:, :, :, 0:15]


def _build(nch=NCH, af=None):
    af = AF.Silu if af is None else af
    nc = bacc.Bacc("TRN2", target_bir_lowering=False, debug=False)

    xt12_d = nc.dram_tensor("xt12", [4, 6, TOT], F32R, kind="ExternalInput").ap()
    zeros_d = nc.dram_tensor("zeros", [128, TALLB], F32R, kind="ExternalInput").ap()
    wd0_d = nc.dram_tensor("wd0", [6, 128], F32R, kind="ExternalInput").ap()
    wdc_d = nc.dram_tensor("wdc", [128, 12, 128], F32R, kind="ExternalInput").ap()
    wpx_d = nc.dram_tensor("wpx", [128, NB, 128], F32R, kind="ExternalInput").ap()
    wc1_d = nc.dram_tensor("wc1", [128, 128], F32R, kind="ExternalInput").ap()
    wc2_d = nc.dram_tensor("wc2", [128, 128], F32R, kind="ExternalInput").ap()
    wf_d = nc.dram_tensor("wf", [128, 64], F32R, kind="ExternalInput").ap()
    bd0_d = nc.dram_tensor("bd0", [128, 1], F32, kind="ExternalInput").ap()
    bdc_d = nc.dram_tensor("bdc", [128, NB], F32, kind="ExternalInput").ap()
    bpx_d = nc.dram_tensor("bpx", [128, NB], F32, kind="ExternalInput").ap()
    bc1_d = nc.dram_tensor("bc1", [128, 1], F32, kind="ExternalInput").ap()
    bc2_d = nc.dram_tensor("bc2", [128, 1], F32, kind="ExternalInput").ap()
    bf_d = nc.dram_tensor("bf", [64, 1], F32, kind="ExternalInput").ap()
    out_d = nc.dram_tensor("out", [BLOC, 4, COUT, 240], F32,
                           kind="ExternalOutput")

    with tile.TileContext(nc) as tc:
        # persistent SBUF tensors
        xsb = [nc.alloc_sbuf_tensor(f"xs{i}", [128, TALLB], F32R).ap()
               for i in range(2)]
        t1 = nc.alloc_sbuf_tensor("t1", [128, TALLB], F32R).ap()
        t2 = nc.alloc_sbuf_tensor("t2", [128, TALLB], F32R).ap()
        xtb = [nc.alloc_sbuf_tensor(f"xt{i}", [128, TALLB], F32R).ap()
               for i in range(2)]
        ofb = [nc.alloc_sbuf_tensor(f"of{i}", [64, NS * 240], F32).ap()
               for i in range(2)]
        zz = nc.alloc_sbuf_tensor("zz", [128, TALLB], F32R).ap()
        wd0 = nc.alloc_sbuf_tensor("wd0s", [128, 128], F32R).ap()
        wdc = nc.alloc_sbuf_tensor("wdcs", [128, 12, 128], F32R).ap()
        wpx = nc.alloc_sbuf_tensor("wpxs", [128, NB, 128], F32R).ap()
        wc1 = nc.alloc_sbuf_tensor("wc1s", [128, 128], F32R).ap()
        wc2 = nc.alloc_sbuf_tensor("wc2s", [128, 128], F32R).ap()
        wf = nc.alloc_sbuf_tensor("wfs", [128, 64], F32R).ap()
        bd0 = nc.alloc_sbuf_tensor("bd0s", [128, 1], F32).ap()
        bdc = nc.alloc_sbuf_tensor("bdcs", [128, NB], F32).ap()
        bpx = nc.alloc_sbuf_tensor("bpxs", [128, NB], F32).ap()
        bc1 = nc.alloc_sbuf_tensor("bc1s", [128, 1], F32).ap()
        bc2 = nc.alloc_sbuf_tensor("bc2s", [128, 1], F32).ap()
        bf = nc.alloc_sbuf_tensor("bfs", [64, 1], F32).ap()

        with tc.tile_pool(name="psum", bufs=2, space="PSUM") as pspool:
            # one-time loads
            nc.sync.dma_start(zz[:], zeros_d)
            for xs in xsb:
                nc.sync.dma_start(xs[:], zeros_d)
            for d in range(4):
                nc.sync.dma_start(wd0[32 * d:32 * d + 6, 0:128], wd0_d)
            nc.sync.dma_start(wdc[:], wdc_d)
            nc.sync.dma_start(wpx[:], wpx_d)
            nc.sync.dma_start(wc1[:], wc1_d)
            nc.sync.dma_start(wc2[:], wc2_d)
            nc.sync.dma_start(wf[:], wf_d)
            for t_, d_ in [(bd0, bd0_d), (bdc, bdc_d), (bpx, bpx_d),
                           (bc1, bc1_d), (bc2, bc2_d), (bf, bf_d)]:
                nc.sync.dma_start(t_[:], d_)

            def conv_layer(src, wap, bias_ap, dst, deltas, psl=None,
                           tile_position=None):
                """3-tap (or 1-tap) conv + silu: src tall -> dst tall.
                wap: lhsT [K, 128]; deltas: list of flat shifts."""
                for gt in range(2):
                    ps = pspool.tile([128, 4, 512], F32, tag="ps")
                    for g in range(4):
                        s0 = 8 * gt + 2 * g
                        for ti, dl in enumerate(deltas):
                            nc.tensor.matmul(
                                ps[:, g, 0:480],
                                wap if len(deltas) == 1 else wap[:, ti, :],
                                _flat240(src, s0, 2, dl, psl=psl),
                                start=(ti == 0), stop=(ti == len(deltas) - 1),
                                tile_position=tile_position,
                            )
                    nc.scalar.activation(
                        _flat240(dst, 8 * gt, 8).rearrange(
                            "p (g s) q -> p g s q", s=2),
                        ps[:, :, 0:480].rearrange("p b (s q) -> p b s q", q=240),
                        af, bias=bias_ap,
                    )

            for ch in range(nch):
                xt = xtb[ch % 2]
                a0 = 256 * NS * ch
                for d in range(4):
                    nc.sync.dma_start(xt[32 * d:32 * d + 6, :],
                                      xt12_d[d, :, a0:a0 + TALLB])
                for d in range(4):
                    xs = xsb[(ch * 4 + d) % 2]
                    deltas = [16 * dy + dx for (dy, dx) in DIR_OFFSETS[d]]

                    # initial conv: K=6 packed taps at partitions 32d
                    psl = slice(32 * d, 32 * d + 6)
                    conv_layer(xt, wd0[psl, 0:128], bd0[:], xs, [0], psl=psl,
                               tile_position=(32 * d, 0))
                    # re-zero xs border col0 (ACT wrote full 240 ranges)
                    bview = lambda buf: buf[:, BLK + 16:BLK + 16 + NS * BLK]\
                        .rearrange("p (s q) -> p s q", q=BLK)[:, :, 0:240:16]
                    nc.vector.tensor_copy(bview(xs), bview(zz))

                    for i in range(NB):
                        conv_layer(xs, wdc[:, 3 * i:3 * i + 3, :],
                                   bdc[:, i:i + 1], t1, deltas)
                        conv_layer(t1, wpx[:, i, :], bpx[:, i:i + 1], t2, [0])
                        nc.vector.tensor_add(_interior(xs), _interior(xs),
                                             _interior(t2))
                    conv_layer(xs, wc1[:], bc1[:], t1, [0])
                    conv_layer(t1, wc2[:], bc2[:], t2, [0])
                    nc.vector.tensor_add(_interior(xs), _interior(xs),
                                         _interior(t2))

                    # final conv (COUT=64) + bias, pack to ofinal
                    of = ofb[(ch * 4 + d) % 2]
                    for gt in range(2):
                        ps = pspool.tile([128, 4, 512], F32, tag="ps")
                        for g in range(4):
                            s0 = 8 * gt + 2 * g
                            nc.tensor.matmul(
                                ps[0:64, g, 0:480], wf[:],
                                _flat240(xs, s0, 2), start=True, stop=True)
                        nc.vector.tensor_scalar_add(
                            of[:, 1920 * gt:1920 * gt + 1920].rearrange(
                                "p (b q) -> p b q", q=480),
                            ps[0:64, :, 0:480], bf[:])

                    src = of[:].rearrange("p (s q) -> p s q", q=240)
                    dst = out_d.ap()[NS * ch:NS * ch + NS, d]\
                        .transpose((1, 0, 2))
                    nc.sync.dma_start(dst, src)

    nc.compile()
    return nc


def _prep(x, w_d0, b_d0, w_dc, b_dc, w_px, b_px, w_c1, b_c1, w_c2, b_c2,
          w_f, b_f):
    """Host-side packing: weights transposed to lhsT, x pre-shifted per
    direction/tap into the tall layout."""
    x = np.asarray(x, np.float32)

    # tall per-core x: [core, 2, TOT]
    xtall = np.zeros((NCORES, CIN, BLOC + 2, 16, 16), np.float32)
    xs = x.reshape(NCORES, BLOC, CIN, H, W)
    xtall[:, :, 1:BLOC + 1, 1:16, 1:16] = xs.transpose(0, 2, 1, 3, 4)
    xtall = xtall.reshape(NCORES, CIN, TOT)

    xt12 = np.zeros((NCORES, 4, 6, TOT), np.float32)
    for d in range(4):
        for t in range(3):
            dy, dx = DIR_OFFSETS[d][t]
            dl = 16 * dy + dx
            for c in range(CIN):
                srcv = xtall[:, c]
                dst = xt12[:, d, 2 * t + c]
                if dl > 0:
                    dst[:, :-dl] = srcv[:, dl:]
                elif dl < 0:
                    dst[:, -dl:] = srcv[:, :dl]
                else:
                    dst[:] = srcv

    com = dict(
        zeros=np.zeros((128, TALLB), np.float32),
        wd0=np.ascontiguousarray(
            np.asarray(w_d0, np.float32).transpose(0, 2, 1).reshape(6, 128)),
        wdc=np.ascontiguousarray(
            np.asarray(w_dc, np.float32).transpose(3, 0, 1, 2).reshape(128, 12, 128)),
        wpx=np.ascontiguousarray(np.asarray(w_px, np.float32).transpose(2, 0, 1)),
        wc1=np.ascontiguousarray(np.asarray(w_c1, np.float32).T),
        wc2=np.ascontiguousarray(np.asarray(w_c2, np.float32).T),
        wf=np.ascontiguousarray(np.asarray(w_f, np.float32).T),
        bd0=np.asarray(b_d0, np.float32).reshape(128, 1),
        bdc=np.ascontiguousarray(np.asarray(b_dc, np.float32).T),
        bpx=np.ascontiguousarray(np.asarray(b_px, np.float32).T),
        bc1=np.asarray(b_c1, np.float32).reshape(128, 1),
        bc2=np.asarray(b_c2, np.float32).reshape(128, 1),
        bf=np.asarray(b_f, np.float32).reshape(64, 1),
    )
    in_maps = []
    for core in range(NCORES):
        m = dict(com)
        m["xt12"] = np.ascontiguousarray(xt12[core])
        in_maps.append(m)
    return in_maps


LAST_RESULT = None


def kernel(**inputs) -> np.ndarray:
    global LAST_RESULT
    if "nc" not in _CACHE:
        _CACHE["nc"] = _build()
    nc = _CACHE["nc"]
    in_maps = _prep(**inputs)
    res = bass_utils.run_bass_kernel_spmd(nc, in_maps,
                                          core_ids=list(range(NCORES)))
    LAST_RESULT = res
    out = np.concatenate([r["out"] for r in res.results], axis=0)
    # strip the border column: 240-flat = 15 rows x 16 cols, col 0 = junk
    out = out.reshape(B, 4, COUT, H, 16)[:, :, :, :, 1:16]
    return np.ascontiguousarray(out)


# revision 11
# speedup vs baseline: 1.4194x; 1.4194x over previous
"""Trainium2 Bass kernel for nn_Mix8Net (dense directional-conv CNN).

Data-parallel over 8 NeuronCores: batch 1024 -> 128 samples/core.

Per-core dataflow (channels on SBUF partitions, pixels*samples on free dim):
  - Activations live in a "tall" layout: per-sample block = 16 rows x 16
    cols = 256 fp32 (row 0 = zero separator, col 0 = zero border, 15x15
    image at rows 1-15 / cols 1-15).  A 3-tap directional conv tap with
    offset (dy,dx) is then a matmul whose moving operand is the
    per-sample flat range [block+16, block+256) shifted by 16*dy+dx —
    out-of-image reads land on the zero separators/borders.
  - All matmuls run in float32r (TF32-like, 1 cycle/row on the PE,
    ~1.5e-4 relative error) accumulating fp32 in PSUM.  fp32r requires
    even innermost AP counts, hence the 240-wide (15 rows x 16 cols)
    moving ranges; the extra border column produces garbage output
    columns that stay column-isolated through the whole network and are
    stripped on the final DMA.
  - Silu runs on ScalarE straight out of PSUM (4 banks / 1920 cols per
    instruction); residual adds run on VectorE over interior-only 4D
    views so the zero borders of the tall buffers are never corrupted.
  - The initial conv (CIN=2, 3 taps) is packed into one K=6 matmul per
    sample pair using host-prepared pre-shifted copies of x, placed at
    partitions 32*d (one group per direction).
"""
import numpy as np
import concourse.bacc as bacc
import concourse.mybir as mybir
import concourse.tile as tile
from concourse import bass_utils

F32 = mybir.dt.float32
F32R = mybir.dt.float32r
AF = mybir.ActivationFunctionType

DIR_OFFSETS = (
    ((0, -1), (0, 0), (0, 1)),
    ((-1, 0), (0, 0), (1, 0)),
    ((-1, -1), (0, 0), (1, 1)),
    ((1, -1), (0, 0), (-1, 1)),
)

B, CIN, H, W = 1024, 2, 15, 15
M, COUT = 128, 64
NB = 4                      # DirectionalConvResBlocks
NCORES = 8
BLOC = B // NCORES          # 128 samples per core
NS = 16                     # samples per chunk
NCH = BLOC // NS            # 8 chunks
BLK = 256                   # 16x16 per-sample block
TALLB = (NS + 2) * BLK      # chunk window incl lead/trail blocks
TOT = (BLOC + 2) * BLK      # full-core tall array length

_CACHE = {}


def _flat240(buf, s0, nsamp, delta=0, psl=None):
    """[p, nsamp(256), 240] moving/store view starting at sample s0."""
    a = 256 * (s0 + 1) + 16 + delta
    v = buf[:, a:a + nsamp * 256] if psl is None else buf[psl, a:a + nsamp * 256]
    return v.rearrange("p (s q) -> p s q", q=BLK)[:, :, 0:240]


def _interior(buf, s0=0, nsamp=NS):
    """[p, nsamp(256), 15(16), 15(1)] interior-pixel view from sample s0."""
    a = BLK * (s0 + 1) + 17
    v = buf[:, a:a + nsamp * BLK]
    v = v.rearrange("p (s q) -> p s q", q=BLK)[:, :, 0:240]
    return v.rearrange("p s (r c) -> p s r c", c=16)[:, :, :, 0:15]


def _build(nch=NCH, af=None):
    af = AF.Silu if af is None else af
    nc = bacc.Bacc("TRN2", target_bir_lowering=False, debug=False)

    xt12_d = nc.dram_tensor("xt12", [4, 6, TOT], F32R, kind="ExternalInput").ap()
    zeros_d = nc.dram_tensor("zeros", [128, TALLB], F32R, kind="ExternalInput").ap()
    wd0_d = nc.dram_tensor("wd0", [6, 128], F32R, kind="ExternalInput").ap()
    wdc_d = nc.dram_tensor("wdc", [128, 12, 128], F32R, kind="ExternalInput").ap()
    wpx_d = nc.dram_tensor("wpx", [128, NB, 128], F32R, kind="ExternalInput").ap()
    wc1_d = nc.dram_tensor("wc1", [128, 128], F32R, kind="ExternalInput").ap()
    wc2_d = nc.dram_tensor("wc2", [128, 128], F32R, kind="ExternalInput").ap()
    wf_d = nc.dram_tensor("wf", [128, 64], F32R, kind="ExternalInput").ap()
    bd0_d = nc.dram_tensor("bd0", [128, 1], F32, kind="ExternalInput").ap()
    bdc_d = nc.dram_tensor("bdc", [128, NB], F32, kind="ExternalInput").ap()
    bpx_d = nc.dram_tensor("bpx", [128, NB], F32, kind="ExternalInput").ap()
    bc1_d = nc.dram_tensor("bc1", [128, 1], F32, kind="ExternalInput").ap()
    bc2_d = nc.dram_tensor("bc2", [128, 1], F32, kind="ExternalInput").ap()
    bf_d = nc.dram_tensor("bf", [64, 1], F32, kind="ExternalInput").ap()
    out_d = nc.dram_tensor("out", [BLOC, 4, COUT, 240], F32,
                           kind="ExternalOutput")

    with tile.TileContext(nc) as tc:
        # persistent SBUF tensors
        xsb = [nc.alloc_sbuf_tensor(f"xs{i}", [128, TALLB], F32R).ap()
               for i in range(2)]
        t1b = [nc.alloc_sbuf_tensor(f"t1{i}", [128, TALLB], F32R).ap()
               for i in range(2)]
        t2b = [nc.alloc_sbuf_tensor(f"t2{i}", [128, TALLB], F32R).ap()
               for i in range(2)]
        xt = nc.alloc_sbuf_tensor("xt", [128, TALLB], F32R).ap()
        ofb = [nc.alloc_sbuf_tensor(f"of{i}", [64, NS * 240], F32).ap()
               for i in range(2)]
        zz = nc.alloc_sbuf_tensor("zz", [128, TALLB], F32R).ap()
        wd0 = nc.alloc_sbuf_tensor("wd0s", [128, 128], F32R).ap()
        wdc = nc.alloc_sbuf_tensor("wdcs", [128, 12, 128], F32R).ap()
        wpx = nc.alloc_sbuf_tensor("wpxs", [128, NB, 128], F32R).ap()
        wc1 = nc.alloc_sbuf_tensor("wc1s", [128, 128], F32R).ap()
        wc2 = nc.alloc_sbuf_tensor("wc2s", [128, 128], F32R).ap()
        wf = nc.alloc_sbuf_tensor("wfs", [128, 64], F32R).ap()
        bd0 = nc.alloc_sbuf_tensor("bd0s", [128, 1], F32).ap()
        bdc = nc.alloc_sbuf_tensor("bdcs", [128, NB], F32).ap()
        bpx = nc.alloc_sbuf_tensor("bpxs", [128, NB], F32).ap()
        bc1 = nc.alloc_sbuf_tensor("bc1s", [128, 1], F32).ap()
        bc2 = nc.alloc_sbuf_tensor("bc2s", [128, 1], F32).ap()
        bf = nc.alloc_sbuf_tensor("bfs", [64, 1], F32).ap()

        with tc.tile_pool(name="psum", bufs=2, space="PSUM") as pspool:
            # one-time loads
            nc.sync.dma_start(zz[:], zeros_d)
            for xs in xsb:
                nc.sync.dma_start(xs[:], zeros_d)
            for d in range(4):
                nc.sync.dma_start(wd0[32 * d:32 * d + 6, 0:128], wd0_d)
            nc.sync.dma_start(wdc[:], wdc_d)
            nc.sync.dma_start(wpx[:], wpx_d)
            nc.sync.dma_start(wc1[:], wc1_d)
            nc.sync.dma_start(wc2[:], wc2_d)
            nc.sync.dma_start(wf[:], wf_d)
            for t_, d_ in [(bd0, bd0_d), (bdc, bdc_d), (bpx, bpx_d),
                           (bc1, bc1_d), (bc2, bc2_d), (bf, bf_d)]:
                nc.sync.dma_start(t_[:], d_)

            def conv_layer(src, wap, bias_ap, dst, deltas, psl=None,
                           tile_position=None):
                """3-tap (or 1-tap) conv + silu: src tall -> dst tall.
                wap: lhsT [K, 128]; deltas: list of flat shifts."""
                for gt in range(2):
                    ps = pspool.tile([128, 4, 512], F32, tag="ps")
                    for g in range(4):
                        s0 = 8 * gt + 2 * g
                        for ti, dl in enumerate(deltas):
                            nc.tensor.matmul(
                                ps[:, g, 0:480],
                                wap if len(deltas) == 1 else wap[:, ti, :],
                                _flat240(src, s0, 2, dl, psl=psl),
                                start=(ti == 0), stop=(ti == len(deltas) - 1),
                                tile_position=tile_position,
                            )
                    nc.scalar.activation(
                        _flat240(dst, 8 * gt, 8).rearrange(
                            "p (g s) q -> p g s q", s=2),
                        ps[:, :, 0:480].rearrange("p b (s q) -> p b s q", q=240),
                        af, bias=bias_ap,
                    )

            def res_add(xs, t2, gt):
                nc.vector.tensor_add(_interior(xs, 8 * gt, 8),
                                     _interior(xs, 8 * gt, 8),
                                     _interior(t2, 8 * gt, 8))

            for ch in range(nch):
                a0 = 256 * NS * ch
                for d in range(4):
                    nc.sync.dma_start(xt[32 * d:32 * d + 6, :],
                                      xt12_d[d, :, a0:a0 + TALLB])
                # interleave direction pairs at layer granularity so one
                # direction's PE bursts cover the other's ACT/DVE latency
                for pair in ((0, 1), (2, 3)):
                    sl = {d: i for i, d in enumerate(pair)}
                    for d in pair:
                        xs = xsb[sl[d]]
                        psl = slice(32 * d, 32 * d + 6)
                        conv_layer(xt, wd0[psl, 0:128], bd0[:], xs, [0],
                                   psl=psl, tile_position=(32 * d, 0))
                        # re-zero xs border col0 (ACT wrote full 240 ranges)
                        bview = lambda buf: \
                            buf[:, BLK + 16:BLK + 16 + NS * BLK].rearrange(
                                "p (s q) -> p s q", q=BLK)[:, :, 0:240:16]
                        nc.vector.tensor_copy(bview(xs), bview(zz))
                    for i in range(NB):
                        for d in pair:
                            deltas = [16 * dy + dx
                                      for (dy, dx) in DIR_OFFSETS[d]]
                            conv_layer(xsb[sl[d]], wdc[:, 3 * i:3 * i + 3, :],
                                       bdc[:, i:i + 1], t1b[sl[d]], deltas)
                        for d in pair:
                            conv_layer(t1b[sl[d]], wpx[:, i, :],
                                       bpx[:, i:i + 1], t2b[sl[d]], [0])
                            for gt in range(2):
                                res_add(xsb[sl[d]], t2b[sl[d]], gt)
                    for d in pair:
                        conv_layer(xsb[sl[d]], wc1[:], bc1[:], t1b[sl[d]], [0])
                    for d in pair:
                        conv_layer(t1b[sl[d]], wc2[:], bc2[:], t2b[sl[d]], [0])
                        for gt in range(2):
                            res_add(xsb[sl[d]], t2b[sl[d]], gt)
                    for d in pair:
                        # final conv (COUT=64) + bias, pack to ofinal
                        xs = xsb[sl[d]]
                        of = ofb[sl[d]]
                        for gt in range(2):
                            ps = pspool.tile([128, 4, 512], F32, tag="ps")
                            for g in range(4):
                                s0 = 8 * gt + 2 * g
                                nc.tensor.matmul(
                                    ps[0:64, g, 0:480], wf[:],
                                    _flat240(xs, s0, 2), start=True, stop=True)
                            nc.vector.tensor_scalar_add(
                                of[:, 1920 * gt:1920 * gt + 1920].rearrange(
                                    "p (b q) -> p b q", q=480),
                                ps[0:64, :, 0:480], bf[:])
                        src = of[:].rearrange("p (s q) -> p s q", q=240)
                        dst = out_d.ap()[NS * ch:NS * ch + NS, d]\
                            .transpose((1, 0, 2))
                        nc.sync.dma_start(dst, src)

    nc.compile()
    return nc


def _prep(x, w_d0, b_d0, w_dc, b_dc, w_px, b_px, w_c1, b_c1, w_c2, b_c2,
          w_f, b_f):
    """Host-side packing: weights transposed to lhsT, x pre-shifted per
    direction/tap into the tall layout."""
    x = np.asarray(x, np.float32)

    # tall per-core x: [core, 2, TOT]
    xtall = np.zeros((NCORES, CIN, BLOC + 2, 16, 16), np.float32)
    xs = x.reshape(NCORES, BLOC, CIN, H, W)
    xtall[:, :, 1:BLOC + 1, 1:16, 1:16] = xs.transpose(0, 2, 1, 3, 4)
    xtall = xtall.reshape(NCORES, CIN, TOT)

    xt12 = np.zeros((NCORES, 4, 6, TOT), np.float32)
    for d in range(4):
        for t in range(3):
            dy, dx = DIR_OFFSETS[d][t]
            dl = 16 * dy + dx
            for c in range(CIN):
                srcv = xtall[:, c]
                dst = xt12[:, d, 2 * t + c]
                if dl > 0:
                    dst[:, :-dl] = srcv[:, dl:]
                elif dl < 0:
                    dst[:, -dl:] = srcv[:, :dl]
                else:
                    dst[:] = srcv

    com = dict(
        zeros=np.zeros((128, TALLB), np.float32),
        wd0=np.ascontiguousarray(
            np.asarray(w_d0, np.float32).transpose(0, 2, 1).reshape(6, 128)),
        wdc=np.ascontiguousarray(
            np.asarray(w_dc, np.float32).transpose(3, 0, 1, 2).reshape(128, 12, 128)),
        wpx=np.ascontiguousarray(np.asarray(w_px, np.float32).transpose(2, 0, 1)),
        wc1=np.ascontiguousarray(np.asarray(w_c1, np.float32).T),
        wc2=np.ascontiguousarray(np.asarray(w_c2, np.float32).T),
        wf=np.ascontiguousarray(np.asarray(w_f, np.float32).T),
        bd0=np.asarray(b_d0, np.float32).reshape(128, 1),
        bdc=np.ascontiguousarray(np.asarray(b_dc, np.float32).T),
        bpx=np.ascontiguousarray(np.asarray(b_px, np.float32).T),
        bc1=np.asarray(b_c1, np.float32).reshape(128, 1),
        bc2=np.asarray(b_c2, np.float32).reshape(128, 1),
        bf=np.asarray(b_f, np.float32).reshape(64, 1),
    )
    in_maps = []
    for core in range(NCORES):
        m = dict(com)
        m["xt12"] = np.ascontiguousarray(xt12[core])
        in_maps.append(m)
    return in_maps


LAST_RESULT = None


def kernel(**inputs) -> np.ndarray:
    global LAST_RESULT
    if "nc" not in _CACHE:
        _CACHE["nc"] = _build()
    nc = _CACHE["nc"]
    in_maps = _prep(**inputs)
    res = bass_utils.run_bass_kernel_spmd(nc, in_maps,
                                          core_ids=list(range(NCORES)))
    LAST_RESULT = res
    out = np.concatenate([r["out"] for r in res.results], axis=0)
    # strip the border column: 240-flat = 15 rows x 16 cols, col 0 = junk
    out = out.reshape(B, 4, COUT, H, 16)[:, :, :, :, 1:16]
    return np.ascontiguousarray(out)


# revision 13
# speedup vs baseline: 1.5738x; 1.1088x over previous
"""Trainium2 Bass kernel for nn_Mix8Net (dense directional-conv CNN).

Data-parallel over 8 NeuronCores: batch 1024 -> 128 samples/core.

Per-core dataflow (channels on SBUF partitions, pixels*samples on free dim):
  - Activations live in a "tall" layout: per-sample block = 16 rows x 16
    cols = 256 fp32 (row 0 = zero separator, col 0 = zero border, 15x15
    image at rows 1-15 / cols 1-15).  A 3-tap directional conv tap with
    offset (dy,dx) is then a matmul whose moving operand is the
    per-sample flat range [block+16, block+256) shifted by 16*dy+dx —
    out-of-image reads land on the zero separators/borders.
  - All matmuls run in float32r (TF32-like, 1 cycle/row on the PE,
    ~1.5e-4 relative error) accumulating fp32 in PSUM.  fp32r requires
    even innermost AP counts, hence the 240-wide (15 rows x 16 cols)
    moving ranges; the extra border column produces garbage output
    columns that stay column-isolated through the whole network and are
    stripped on the final DMA.
  - Silu runs on ScalarE straight out of PSUM (4 banks / 1920 cols per
    instruction); residual adds run on VectorE over interior-only 4D
    views so the zero borders of the tall buffers are never corrupted.
  - The initial conv (CIN=2, 3 taps) is packed into one K=6 matmul per
    sample pair using host-prepared pre-shifted copies of x, placed at
    partitions 32*d (one group per direction).
"""
import numpy as np
import concourse.bacc as bacc
import concourse.mybir as mybir
import concourse.tile as tile
from concourse import bass_utils

F32 = mybir.dt.float32
F32R = mybir.dt.float32r
AF = mybir.ActivationFunctionType

DIR_OFFSETS = (
    ((0, -1), (0, 0), (0, 1)),
    ((-1, 0), (0, 0), (1, 0)),
    ((-1, -1), (0, 0), (1, 1)),
    ((1, -1), (0, 0), (-1, 1)),
)

B, CIN, H, W = 1024, 2, 15, 15
M, COUT = 128, 64
NB = 4                      # DirectionalConvResBlocks
NCORES = 8
BLOC = B // NCORES          # 128 samples per core
NS = 16                     # samples per chunk
NCH = BLOC // NS            # 8 chunks
BLK = 256                   # 16x16 per-sample block
TALLB = (NS + 2) * BLK      # chunk window incl lead/trail blocks
TOT = (BLOC + 2) * BLK      # full-core tall array length

_CACHE = {}


def _flat240(buf, s0, nsamp, delta=0, psl=None):
    """[p, nsamp(256), 240] moving/store view starting at sample s0."""
    a = 256 * (s0 + 1) + 16 + delta
    v = buf[:, a:a + nsamp * 256] if psl is None else buf[psl, a:a + nsamp * 256]
    return v.rearrange("p (s q) -> p s q", q=BLK)[:, :, 0:240]


def _interior(buf, s0=0, nsamp=NS):
    """[p, nsamp(256), 15(16), 15(1)] interior-pixel view from sample s0."""
    a = BLK * (s0 + 1) + 17
    v = buf[:, a:a + nsamp * BLK]
    v = v.rearrange("p (s q) -> p s q", q=BLK)[:, :, 0:240]
    return v.rearrange("p s (r c) -> p s r c", c=16)[:, :, :, 0:15]


def _build(nch=NCH, af=None):
    af = AF.Silu if af is None else af
    nc = bacc.Bacc("TRN2", target_bir_lowering=False, debug=False)

    xt12_d = nc.dram_tensor("xt12", [4, 6, TOT], F32R, kind="ExternalInput").ap()
    zeros_d = nc.dram_tensor("zeros", [128, TALLB], F32R, kind="ExternalInput").ap()
    wd0_d = nc.dram_tensor("wd0", [6, 128], F32R, kind="ExternalInput").ap()
    wdc_d = nc.dram_tensor("wdc", [128, 12, 128], F32R, kind="ExternalInput").ap()
    wpx_d = nc.dram_tensor("wpx", [128, NB, 128], F32R, kind="ExternalInput").ap()
    wc1_d = nc.dram_tensor("wc1", [128, 128], F32R, kind="ExternalInput").ap()
    wc2_d = nc.dram_tensor("wc2", [128, 128], F32R, kind="ExternalInput").ap()
    wf_d = nc.dram_tensor("wf", [128, 64], F32R, kind="ExternalInput").ap()
    bd0_d = nc.dram_tensor("bd0", [128, 1], F32, kind="ExternalInput").ap()
    bdc_d = nc.dram_tensor("bdc", [128, NB], F32, kind="ExternalInput").ap()
    bpx_d = nc.dram_tensor("bpx", [128, NB], F32, kind="ExternalInput").ap()
    bc1_d = nc.dram_tensor("bc1", [128, 1], F32, kind="ExternalInput").ap()
    bc2_d = nc.dram_tensor("bc2", [128, 1], F32, kind="ExternalInput").ap()
    bf_d = nc.dram_tensor("bf", [64, 1], F32, kind="ExternalInput").ap()
    out_d = nc.dram_tensor("out", [BLOC, 4, COUT, 240], F32,
                           kind="ExternalOutput")

    with tile.TileContext(nc) as tc:
        # persistent SBUF tensors
        xsb = [nc.alloc_sbuf_tensor(f"xs{i}", [128, TALLB], F32R).ap()
               for i in range(2)]
        t1b = [nc.alloc_sbuf_tensor(f"t1{i}", [128, TALLB], F32R).ap()
               for i in range(2)]
        t2b = [nc.alloc_sbuf_tensor(f"t2{i}", [128, TALLB], F32R).ap()
               for i in range(2)]
        xt = nc.alloc_sbuf_tensor("xt", [128, TALLB], F32R).ap()
        ofb = [nc.alloc_sbuf_tensor(f"of{i}", [64, NS * 240], F32).ap()
               for i in range(2)]
        zz = nc.alloc_sbuf_tensor("zz", [128, TALLB], F32R).ap()
        wd0 = nc.alloc_sbuf_tensor("wd0s", [128, 128], F32R).ap()
        wdc = nc.alloc_sbuf_tensor("wdcs", [128, 12, 128], F32R).ap()
        wpx = nc.alloc_sbuf_tensor("wpxs", [128, NB, 128], F32R).ap()
        wc1 = nc.alloc_sbuf_tensor("wc1s", [128, 128], F32R).ap()
        wc2 = nc.alloc_sbuf_tensor("wc2s", [128, 128], F32R).ap()
        wf = nc.alloc_sbuf_tensor("wfs", [128, 64], F32R).ap()
        bd0 = nc.alloc_sbuf_tensor("bd0s", [128, 1], F32).ap()
        bdc = nc.alloc_sbuf_tensor("bdcs", [128, NB], F32).ap()
        bpx = nc.alloc_sbuf_tensor("bpxs", [128, NB], F32).ap()
        bc1 = nc.alloc_sbuf_tensor("bc1s", [128, 1], F32).ap()
        bc2 = nc.alloc_sbuf_tensor("bc2s", [128, 1], F32).ap()
        bf = nc.alloc_sbuf_tensor("bfs", [64, 1], F32).ap()

        with tc.tile_pool(name="psum", bufs=2, space="PSUM") as pspool:
            # one-time loads
            nc.sync.dma_start(zz[:], zeros_d)
            for xs in xsb:
                nc.sync.dma_start(xs[:], zeros_d)
            for d in range(4):
                nc.sync.dma_start(wd0[32 * d:32 * d + 6, 0:128], wd0_d)
            nc.sync.dma_start(wdc[:], wdc_d)
            nc.sync.dma_start(wpx[:], wpx_d)
            nc.sync.dma_start(wc1[:], wc1_d)
            nc.sync.dma_start(wc2[:], wc2_d)
            nc.sync.dma_start(wf[:], wf_d)
            for t_, d_ in [(bd0, bd0_d), (bdc, bdc_d), (bpx, bpx_d),
                           (bc1, bc1_d), (bc2, bc2_d), (bf, bf_d)]:
                nc.sync.dma_start(t_[:], d_)

            def conv_layer(src, wap, bias_ap, dst, deltas, psl=None,
                           tile_position=None):
                """3-tap (or 1-tap) conv + silu: src tall -> dst tall.
                wap: lhsT [K, 128]; deltas: list of flat shifts."""
                for gt in range(2):
                    ps = pspool.tile([128, 4, 512], F32, tag="ps")
                    for g in range(4):
                        s0 = 8 * gt + 2 * g
                        for ti, dl in enumerate(deltas):
                            nc.tensor.matmul(
                                ps[:, g, 0:480],
                                wap if len(deltas) == 1 else wap[:, ti, :],
                                _flat240(src, s0, 2, dl, psl=psl),
                                start=(ti == 0), stop=(ti == len(deltas) - 1),
                                tile_position=tile_position,
                            )
                        if g % 2 == 1:
                            # drain the filled bank pair while the PE fills
                            # the next one — finer PSUM hand-back
                            h = g // 2
                            nc.scalar.activation(
                                _flat240(dst, 8 * gt + 4 * h, 4).rearrange(
                                    "p (g s) q -> p g s q", s=2),
                                ps[:, 2 * h:2 * h + 2, 0:480].rearrange(
                                    "p b (s q) -> p b s q", q=240),
                                af, bias=bias_ap,
                            )

            def res_add(xs, t2, gt):
                nc.vector.tensor_add(_interior(xs, 8 * gt, 8),
                                     _interior(xs, 8 * gt, 8),
                                     _interior(t2, 8 * gt, 8))

            for ch in range(nch):
                a0 = 256 * NS * ch
                for d in range(4):
                    nc.sync.dma_start(xt[32 * d:32 * d + 6, :],
                                      xt12_d[d, :, a0:a0 + TALLB])
                # interleave direction pairs at layer granularity so one
                # direction's PE bursts cover the other's ACT/DVE latency
                for pair in ((0, 1), (2, 3)):
                    sl = {d: i for i, d in enumerate(pair)}
                    for d in pair:
                        xs = xsb[sl[d]]
                        psl = slice(32 * d, 32 * d + 6)
                        conv_layer(xt, wd0[psl, 0:128], bd0[:], xs, [0],
                                   psl=psl, tile_position=(32 * d, 0))
                        # re-zero xs border col0 (ACT wrote full 240 ranges)
                        bview = lambda buf: \
                            buf[:, BLK + 16:BLK + 16 + NS * BLK].rearrange(
                                "p (s q) -> p s q", q=BLK)[:, :, 0:240:16]
                        nc.vector.tensor_copy(bview(xs), bview(zz))
                    for i in range(NB):
                        for d in pair:
                            deltas = [16 * dy + dx
                                      for (dy, dx) in DIR_OFFSETS[d]]
                            conv_layer(xsb[sl[d]], wdc[:, 3 * i:3 * i + 3, :],
                                       bdc[:, i:i + 1], t1b[sl[d]], deltas)
                        for d in pair:
                            conv_layer(t1b[sl[d]], wpx[:, i, :],
                                       bpx[:, i:i + 1], t2b[sl[d]], [0])
                            for gt in range(2):
                                res_add(xsb[sl[d]], t2b[sl[d]], gt)
                    for d in pair:
                        conv_layer(xsb[sl[d]], wc1[:], bc1[:], t1b[sl[d]], [0])
                    for d in pair:
                        conv_layer(t1b[sl[d]], wc2[:], bc2[:], t2b[sl[d]], [0])
                        for gt in range(2):
                            res_add(xsb[sl[d]], t2b[sl[d]], gt)
                    for d in pair:
                        # final conv (COUT=64) + bias, pack to ofinal
                        xs = xsb[sl[d]]
                        of = ofb[sl[d]]
                        for gt in range(2):
                            ps = pspool.tile([128, 4, 512], F32, tag="ps")
                            for g in range(4):
                                s0 = 8 * gt + 2 * g
                                nc.tensor.matmul(
                                    ps[0:64, g, 0:480], wf[:],
                                    _flat240(xs, s0, 2), start=True, stop=True)
                                if g % 2 == 1:
                                    h = g // 2
                                    a = 1920 * gt + 960 * h
                                    nc.scalar.activation(
                                        of[:, a:a + 960].rearrange(
                                            "p (b q) -> p b q", q=480),
                                        ps[0:64, 2 * h:2 * h + 2, 0:480],
                                        AF.Identity, bias=bf[:])
                        src = of[:].rearrange("p (s q) -> p s q", q=240)
                        dst = out_d.ap()[NS * ch:NS * ch + NS, d]\
                            .transpose((1, 0, 2))
                        nc.sync.dma_start(dst, src)

    nc.compile()
    return nc


def _prep(x, w_d0, b_d0, w_dc, b_dc, w_px, b_px, w_c1, b_c1, w_c2, b_c2,
          w_f, b_f):
    """Host-side packing: weights transposed to lhsT, x pre-shifted per
    direction/tap into the tall layout."""
    x = np.asarray(x, np.float32)

    # tall per-core x: [core, 2, TOT]
    xtall = np.zeros((NCORES, CIN, BLOC + 2, 16, 16), np.float32)
    xs = x.reshape(NCORES, BLOC, CIN, H, W)
    xtall[:, :, 1:BLOC + 1, 1:16, 1:16] = xs.transpose(0, 2, 1, 3, 4)
    xtall = xtall.reshape(NCORES, CIN, TOT)

    xt12 = np.zeros((NCORES, 4, 6, TOT), np.float32)
    for d in range(4):
        for t in range(3):
            dy, dx = DIR_OFFSETS[d][t]
            dl = 16 * dy + dx
            for c in range(CIN):
                srcv = xtall[:, c]
                dst = xt12[:, d, 2 * t + c]
                if dl > 0:
                    dst[:, :-dl] = srcv[:, dl:]
                elif dl < 0:
                    dst[:, -dl:] = srcv[:, :dl]
                else:
                    dst[:] = srcv

    com = dict(
        zeros=np.zeros((128, TALLB), np.float32),
        wd0=np.ascontiguousarray(
            np.asarray(w_d0, np.float32).transpose(0, 2, 1).reshape(6, 128)),
        wdc=np.ascontiguousarray(
            np.asarray(w_dc, np.float32).transpose(3, 0, 1, 2).reshape(128, 12, 128)),
        wpx=np.ascontiguousarray(np.asarray(w_px, np.float32).transpose(2, 0, 1)),
        wc1=np.ascontiguousarray(np.asarray(w_c1, np.float32).T),
        wc2=np.ascontiguousarray(np.asarray(w_c2, np.float32).T),
        wf=np.ascontiguousarray(np.asarray(w_f, np.float32).T),
        bd0=np.asarray(b_d0, np.float32).reshape(128, 1),
        bdc=np.ascontiguousarray(np.asarray(b_dc, np.float32).T),
        bpx=np.ascontiguousarray(np.asarray(b_px, np.float32).T),
        bc1=np.asarray(b_c1, np.float32).reshape(128, 1),
        bc2=np.asarray(b_c2, np.float32).reshape(128, 1),
        bf=np.asarray(b_f, np.float32).reshape(64, 1),
    )
    in_maps = []
    for core in range(NCORES):
        m = dict(com)
        m["xt12"] = np.ascontiguousarray(xt12[core])
        in_maps.append(m)
    return in_maps


LAST_RESULT = None


def kernel(**inputs) -> np.ndarray:
    global LAST_RESULT
    if "nc" not in _CACHE:
        _CACHE["nc"] = _build()
    nc = _CACHE["nc"]
    in_maps = _prep(**inputs)
    res = bass_utils.run_bass_kernel_spmd(nc, in_maps,
                                          core_ids=list(range(NCORES)))
    LAST_RESULT = res
    out = np.concatenate([r["out"] for r in res.results], axis=0)
    # strip the border column: 240-flat = 15 rows x 16 cols, col 0 = junk
    out = out.reshape(B, 4, COUT, H, 16)[:, :, :, :, 1:16]
    return np.ascontiguousarray(out)
